# revision 36
# baseline (speedup 1.0000x reference)
"""Trainium2 Bass kernel for nn_AggrHGraphConvWindow (3x GraphConv -> LeakyReLU -> 2-layer LSTM).

Contract: kernel(**inputs) takes FULL unsharded numpy inputs, returns FULL output
(33500, 16, 128) float32.  Internally shards destination rows across 8 NeuronCores
(graph/data parallel per the sharding hint), runs one SPMD Bass program, gathers.

Key structural optimization: the pod graph has mean in-degree 1 and the svc graph
mean in-degree 4, so many destination rows are provably identical:
  - in-degree-0 rows:  x0 = LeakyReLU(bias)            -> one shared row
  - in-degree-1 rows:  x0 determined by the single src -> one row per distinct src
The kernel computes each distinct row once and scatters host-side.  This shrinks
the LSTM batch (and conv work) from 33500 rows to ~11900 (1486/core, padded 1536).

Per-core row layout is contiguous [pods | svc | nodes | pad]; conv tiles may mix
node types (per-segment weight matmuls in the tail).  LSTM runs three uniform
512-row batch tiles as independent per-tile recurrence chains so the engines
pipeline across tiles.
"""

import numpy as np
import ml_dtypes

BF16 = np.float16  # fp16: same cost as bf16 on PE/DVE, 8x finer mantissa
FP8 = ml_dtypes.float8_e4m3  # maps to mybir float8e4 (DoubleRow-capable)

# Problem constants (hardcoded per spec)
N_NODE, N_POD, N_SVC = 500, 30000, 3000
T, F, H = 16, 64, 128
NCORES = 8
P = 128
KIND_POD, KIND_SVC, KIND_NODE = 0, 1, 2  # conv phase source kinds

NODE_PC = (N_NODE + NCORES - 1) // NCORES  # 63

_COMPILED = {}


# ----------------------------------------------------------------------------
# Host-side preprocessing: dedup, edge routing, degree norms, weight prep
# ----------------------------------------------------------------------------

def _degrees(src, dst, n_src, n_dst):
    dout = np.bincount(src, minlength=n_src).astype(np.float64)
    din = np.bincount(dst, minlength=n_dst).astype(np.float64)
    return (1.0 / np.sqrt(np.maximum(dout, 1.0)), 1.0 / np.sqrt(np.maximum(din, 1.0)))


def _dedup(src, dst, n_dst, ro, ri):
    """Collapse identical destination rows. Returns (src', dst', w', map, n_unique).
    Row order: deg>=2 dsts (by id), then one row per distinct src of deg-1 dsts,
    then (if any deg-0 dsts) one shared zero-aggregation row."""
    deg = np.bincount(dst, minlength=n_dst)
    w = (ro[src] * ri[dst]).astype(np.float32)
    keep = deg[dst] >= 2
    d2 = np.where(deg >= 2)[0]
    rank = np.zeros(n_dst, np.int64)
    rank[d2] = np.arange(len(d2))
    one = deg[dst] == 1
    u1 = np.unique(src[one])
    mp = np.zeros(n_dst, np.int64)
    mp[d2] = rank[d2]
    mp[dst[one]] = len(d2) + np.searchsorted(u1, src[one])
    nz = len(d2) + len(u1)
    deg0 = np.where(deg == 0)[0]
    n_u = nz + (1 if len(deg0) else 0)
    mp[deg0] = nz
    es = np.concatenate([src[keep], u1]).astype(np.int64)
    ed = np.concatenate([rank[dst[keep]], len(d2) + np.arange(len(u1))]).astype(np.int64)
    # deg-1 dst: ri = 1/sqrt(1) = 1, so the class weight is just ro[src]
    ew = np.concatenate([w[keep], ro[u1].astype(np.float32)]).astype(np.float32)
    return es, ed, ew, mp, n_u


def _prep(inputs):
    nf = np.asarray(inputs["node_feat"]).reshape(N_NODE, T * F)
    pf = np.asarray(inputs["pod_feat"]).reshape(N_POD, T * F)
    sf = np.asarray(inputs["svc_feat"]).reshape(N_SVC, T * F)

    in_src = np.asarray(inputs["inst_node_src"]).astype(np.int64)
    in_dst = np.asarray(inputs["inst_node_dst"]).astype(np.int64)
    ni_src = np.asarray(inputs["node_inst_src"]).astype(np.int64)
    ni_dst = np.asarray(inputs["node_inst_dst"]).astype(np.int64)
    sc_src = np.asarray(inputs["svc_call_src"]).astype(np.int64)
    sc_dst = np.asarray(inputs["svc_call_dst"]).astype(np.int64)

    # normalization: x/sqrt(deg_out) -> segsum -> /sqrt(deg_in), folded per-edge
    ro_in, ri_in = _degrees(in_src, in_dst, N_POD, N_NODE)
    ro_ni, ri_ni = _degrees(ni_src, ni_dst, N_NODE, N_POD)
    ro_sc, ri_sc = _degrees(sc_src, sc_dst, N_SVC, N_SVC)

    # --- destination-row dedup for pod and svc phases ---
    pe_s, pe_d, pe_w, pod_map, n_pod_u = _dedup(ni_src, ni_dst, N_POD, ro_ni, ri_ni)
    se_s, se_d, se_w, svc_map, n_svc_u = _dedup(sc_src, sc_dst, N_SVC, ro_sc, ri_sc)
    ne_s, ne_d = in_src, in_dst
    ne_w = (ro_in[in_src] * ri_in[in_dst]).astype(np.float32)

    POD_PC = (n_pod_u + NCORES - 1) // NCORES
    SVC_PC = (n_svc_u + NCORES - 1) // NCORES
    ROWS = POD_PC + SVC_PC + NODE_PC
    R_CORE = 512 * ((ROWS + 511) // 512)
    N_TILES = R_CORE // P
    TYPE_PC = {KIND_POD: POD_PC, KIND_SVC: SVC_PC, KIND_NODE: NODE_PC}
    TYPE_BASE = {KIND_POD: 0, KIND_SVC: POD_PC, KIND_NODE: POD_PC + SVC_PC}

    # Route edges into per (core, tile, kind) buckets.
    def route(src, dst, w, kind):
        pc = TYPE_PC[kind]
        core = dst // pc
        row = TYPE_BASE[kind] + (dst - core * pc)
        return core, row // P, row % P, src, w

    routed = {
        KIND_POD: route(pe_s, pe_d, pe_w, KIND_POD),    # src = nodes (one-hot)
        KIND_SVC: route(se_s, se_d, se_w, KIND_SVC),    # src = svcs (gather)
        KIND_NODE: route(ne_s, ne_d, ne_w, KIND_NODE),  # src = pods (gather)
    }

    empty = (np.zeros(0, np.int64), np.zeros(0, np.int64), np.zeros(0, np.float32))
    buckets = [[[empty for _ in range(3)] for _ in range(N_TILES)]
               for _ in range(NCORES)]
    for kind in (KIND_POD, KIND_SVC, KIND_NODE):
        core, tile, row, src, w = routed[kind]
        order = np.lexsort((row, tile, core))
        core, tile, row, src, w = core[order], tile[order], row[order], src[order], w[order]
        key = core * N_TILES + tile
        uniq, starts = np.unique(key, return_index=True)
        starts = list(starts) + [len(key)]
        for ui, k in enumerate(uniq):
            c, t = int(k) // N_TILES, int(k) % N_TILES
            s, e = starts[ui], starts[ui + 1]
            buckets[c][t][kind] = (src[s:e], row[s:e], w[s:e])

    # static chunk counts per (tile, kind): max over cores; >= 1 chunk per tile
    K = []
    for t in range(N_TILES):
        kk = [0, 0, 0]
        for kind in range(3):
            for c in range(NCORES):
                kk[kind] = max(kk[kind],
                               (len(buckets[c][t][kind][0]) + P - 1) // P)
        if sum(kk) == 0:
            kk[KIND_NODE] = 1  # dummy zero-weight chunk keeps agg defined
        K.append(tuple(kk))
    # chunk column layout: per tile, kinds in order pod, svc, node
    base = [0]
    for t in range(N_TILES):
        base.append(base[-1] + sum(K[t]))
    C_total = base[-1]

    PN = sum(k[KIND_NODE] for k in K) * P   # node-phase podtab rows
    PS = sum(k[KIND_SVC] for k in K) * P    # svc-phase svctab rows
    nodetab = np.zeros((512, T * F), dtype=BF16)
    nodetab[:N_NODE] = nf.astype(BF16)

    in_maps = []
    for c in range(NCORES):
        esrc = np.zeros((C_total, P), dtype=np.int32)
        esrcf = np.zeros((C_total, P), dtype=np.float32)
        edst = np.zeros((C_total, P), dtype=np.float32)
        ew = np.zeros((C_total, P), dtype=np.float32)
        podtab = np.zeros((max(PN, P), T * F), dtype=BF16)
        svctab = np.zeros((max(PS, P), T * F), dtype=BF16)
        pod_fill = 0
        svc_fill = 0

        for t in range(N_TILES):
            col = base[t]
            for kind in (KIND_POD, KIND_SVC, KIND_NODE):
                src, row, w = buckets[c][t][kind]
                n = len(src)
                nchunk = K[t][kind]
                if kind == KIND_POD:
                    sidx = src  # direct index into nodetab (one-hot path)
                elif kind == KIND_SVC:
                    u2, i2 = (np.unique(src, return_inverse=True) if n
                              else (np.zeros(0, np.int64), np.zeros(0, np.int64)))
                    svctab[svc_fill:svc_fill + len(u2)] = sf[u2].astype(BF16)
                    sidx = i2 + svc_fill
                    svc_fill += nchunk * P
                else:
                    u2, i2 = (np.unique(src, return_inverse=True) if n
                              else (np.zeros(0, np.int64), np.zeros(0, np.int64)))
                    podtab[pod_fill:pod_fill + len(u2)] = pf[u2].astype(BF16)
                    sidx = i2 + pod_fill
                    pod_fill += nchunk * P
                b0 = col * P
                esrc.reshape(-1)[b0:b0 + n] = sidx
                esrcf.reshape(-1)[b0:b0 + n] = sidx
                edst.reshape(-1)[b0:b0 + n] = row
                ew.reshape(-1)[b0:b0 + n] = w
                col += nchunk

        m = {
            "podtab": podtab, "nodetab": nodetab, "svctab": svctab,
            "esrc": np.ascontiguousarray(esrc.T),
            "esrcf": np.ascontiguousarray(esrcf.T),
            "edst": np.ascontiguousarray(edst.T),
            "ew": np.ascontiguousarray(ew.T),
        }
        in_maps.append(m)

    # ---- weights (identical on all cores) ----
    def conv_w(Wname):
        W = np.asarray(inputs[Wname])  # (T, F, H)
        wt = W.transpose(1, 0, 2).reshape(F, T * H)  # (64, 2048) F-major
        return np.vstack([wt, wt]).astype(BF16)       # (128, 2048) vertical dup

    def conv_b(bname):
        return np.asarray(inputs[bname]).reshape(1, T * H).astype(BF16)

    def lstm_w(Wname, dt=BF16):
        # rows [i,f,g,o] -> [i,f,o,g]; g block doubled so tanh(g) = 2*sigmoid(2g)-1
        W = np.asarray(inputs[Wname])  # (512, in_dim)
        Wp = np.concatenate([W[0:128], W[128:256], W[384:512], 2.0 * W[256:384]], axis=0)
        return np.ascontiguousarray(Wp.T).astype(dt)  # (in_dim, 512), [i,f,o,2g]

    def lstm_w_dr():
        # layer-0 DoubleRow weights, fp8: [Wih (512) | Whh (512)] so a chunk's
        # (Wih_i, Whh_i) pair is a stride-512 [2,128] AP (the DoubleRow
        # stationary layout; PE fuses the K=256 [x;h] contraction)
        wdr = np.zeros((H, 1024), dtype=FP8)
        wdr[:, 0:512] = lstm_w("Wih0", np.float32).astype(FP8)
        wdr[:, 512:1024] = lstm_w("Whh0", np.float32).astype(FP8)
        return wdr

    def lstm_b(b1, b2):
        b = np.asarray(inputs[b1]) + np.asarray(inputs[b2])
        bp = np.concatenate([b[0:128], b[128:256], b[384:512], 2.0 * b[256:384]])
        return bp.reshape(1, 512).astype(BF16)

    shared = {
        "wt_pod": conv_w("W_ni"), "wt_svc": conv_w("W_svc"), "wt_node": conv_w("W_in"),
        "bt_pod": conv_b("b_ni"), "bt_svc": conv_b("b_svc"), "bt_node": conv_b("b_in"),
        "wdr0": lstm_w_dr(),
        "wih1": lstm_w("Wih1"), "whh1": lstm_w("Whh1"),
        "bias0": lstm_b("bih0", "bhh0"), "bias1": lstm_b("bih1", "bhh1"),
        "iota": np.broadcast_to(np.arange(P, dtype=BF16), (P, P)).copy(),
        "iota512": np.broadcast_to(np.arange(512, dtype=BF16), (P, 512)).copy(),
    }
    for m in in_maps:
        m.update(shared)

    meta = (C_total, tuple(K), max(PN, P), max(PS, P), POD_PC, SVC_PC)
    maps = (pod_map, svc_map, n_pod_u, n_svc_u)
    return meta, maps, in_maps


# ----------------------------------------------------------------------------
# Device program
# ----------------------------------------------------------------------------

def _build(meta):
    import concourse.bass as bass
    import concourse.tile as tile
    import concourse.mybir as mybir

    (C_total, K, PN, PS, POD_PC, SVC_PC) = meta
    ROWS = POD_PC + SVC_PC + NODE_PC
    R_CORE = 512 * ((ROWS + 511) // 512)
    N_TILES = R_CORE // P
    NJ = R_CORE // 512
    base = [0]
    for t in range(N_TILES):
        base.append(base[-1] + sum(K[t]))

    # per-tile segments (kind, col0, col1) from the contiguous type layout;
    # the final segment of each tile extends through padding so every row of
    # hx is written before the Prelu.
    bounds = [(KIND_POD, 0, POD_PC), (KIND_SVC, POD_PC, POD_PC + SVC_PC),
              (KIND_NODE, POD_PC + SVC_PC, R_CORE)]
    segments = []
    for t in range(N_TILES):
        lo, hi = t * P, (t + 1) * P
        segs = []
        for kind, b0, b1 in bounds:
            s0, s1 = max(lo, b0), min(hi, b1)
            if s0 < s1:
                segs.append((kind, s0 - lo, s1 - lo))
        if not segs:
            segs.append((KIND_NODE, 0, P))
        else:
            kind, s0, _ = segs[-1]
            segs[-1] = (kind, s0, P)  # cover padding
        segments.append(segs)

    f32 = mybir.dt.float32
    bf16 = mybir.dt.float16
    i32 = mybir.dt.int32
    AF = mybir.ActivationFunctionType
    ALU = mybir.AluOpType

    import concourse.bacc as bacc
    nc = bacc.Bacc("TRN2", target_bir_lowering=False, debug=False, enable_asserts=False)

    podtab = nc.dram_tensor("podtab", [PN, T * F], bf16, kind="ExternalInput")
    nodetab = nc.dram_tensor("nodetab", [512, T * F], bf16, kind="ExternalInput")
    svctab = nc.dram_tensor("svctab", [PS, T * F], bf16, kind="ExternalInput")
    esrc_d = nc.dram_tensor("esrc", [P, C_total], i32, kind="ExternalInput")
    esrcf_d = nc.dram_tensor("esrcf", [P, C_total], f32, kind="ExternalInput")
    edst_d = nc.dram_tensor("edst", [P, C_total], f32, kind="ExternalInput")
    ew_d = nc.dram_tensor("ew", [P, C_total], f32, kind="ExternalInput")
    KINDNM = {KIND_POD: "pod", KIND_SVC: "svc", KIND_NODE: "node"}
    wt_d = {k: nc.dram_tensor(f"wt_{k}", [P, T * H], bf16, kind="ExternalInput")
            for k in ("pod", "svc", "node")}
    bt_d = {k: nc.dram_tensor(f"bt_{k}", [1, T * H], bf16, kind="ExternalInput")
            for k in ("pod", "svc", "node")}
    fp8 = mybir.dt.float8e4
    wdr0_d = nc.dram_tensor("wdr0", [H, 1024], fp8, kind="ExternalInput")
    wih1_d = nc.dram_tensor("wih1", [H, 512], bf16, kind="ExternalInput")
    whh1_d = nc.dram_tensor("whh1", [H, 512], bf16, kind="ExternalInput")
    bias_d = [nc.dram_tensor(f"bias{l}", [1, 512], bf16, kind="ExternalInput") for l in range(2)]
    iota_d = nc.dram_tensor("iota", [P, P], bf16, kind="ExternalInput")
    iota512_d = nc.dram_tensor("iota512", [P, 512], bf16, kind="ExternalInput")
    out_d = nc.dram_tensor("out", [P, T * R_CORE], bf16, kind="ExternalOutput")

    with tile.TileContext(nc) as tc:
        with tc.tile_pool(name="const", bufs=1) as constp:
            esrc_sb = constp.tile([P, C_total], i32)
            esrcf_sb = constp.tile([P, C_total], f32)
            edst_sb = constp.tile([P, C_total], f32)
            ew_sb = constp.tile([P, C_total], f32)
            iota_sb = constp.tile([P, P], bf16)
            iota512_sb = constp.tile([P, 512], bf16)
            nc.sync.dma_start(esrc_sb[:], esrc_d.ap())
            nc.sync.dma_start(esrcf_sb[:], esrcf_d.ap())
            nc.sync.dma_start(edst_sb[:], edst_d.ap())
            nc.sync.dma_start(ew_sb[:], ew_d.ap())
            nc.sync.dma_start(iota_sb[:], iota_d.ap())
            nc.sync.dma_start(iota512_sb[:], iota512_d.ap())
            # nodetab resident in SBUF as 4 chunks of 128 rows (one-hot path)
            ntab_sb = []
            for sc in range(4):
                tt = constp.tile([P, T * F], bf16, name=f"ntab_{sc}")
                nc.sync.dma_start(tt[:], nodetab.ap()[sc * P:(sc + 1) * P, :])
                ntab_sb.append(tt)
            wt_sb, bt_sb = {}, {}
            for k in ("pod", "svc", "node"):
                wt_sb[k] = constp.tile([P, T * H], bf16, name=f"wt_{k}_sb")
                bt_sb[k] = constp.tile([1, T * H], bf16, name=f"bt_{k}_sb")
                nc.sync.dma_start(wt_sb[k][:], wt_d[k].ap())
                nc.sync.dma_start(bt_sb[k][:], bt_d[k].ap())
            wdr0_sb = constp.tile([H, 1024], fp8, name="wdr0_sb")
            wih1_sb = constp.tile([H, 512], bf16, name="wih1_sb")
            whh1_sb = constp.tile([H, 512], bf16, name="whh1_sb")
            nc.sync.dma_start(wdr0_sb[:], wdr0_d.ap())
            nc.sync.dma_start(wih1_sb[:], wih1_d.ap())
            nc.sync.dma_start(whh1_sb[:], whh1_d.ap())
            bias_sb = []
            for l in range(2):
                bias_sb.append(constp.tile([1, 512], bf16, name=f"bias{l}_sb"))
                nc.sync.dma_start(bias_sb[l][:], bias_d[l].ap())
            ones_sb = constp.tile([1, 512], bf16)
            nc.gpsimd.memset(ones_sb[:], 1.0)

            srctabs = {KIND_SVC: svctab, KIND_NODE: podtab}

            with tc.tile_pool(name="gat", bufs=5) as gatp, \
                 tc.tile_pool(name="gatg", bufs=8) as gatgp, \
                 tc.tile_pool(name="ssb", bufs=12) as ssbp, \
                 tc.tile_pool(name="psum", bufs=2, space="PSUM") as psump, \
                 tc.tile_pool(name="aggsb", bufs=2) as aggsbp, \
                 tc.tile_pool(name="x0res", bufs=NJ) as x0resp, \
                 tc.tile_pool(name="hc", bufs=18) as hcp, \
                 tc.tile_pool(name="sg", bufs=4) as sgp, \
                 tc.tile_pool(name="gt", bufs=3) as gtp, \
                 tc.tile_pool(name="t1", bufs=3) as t1p, \
                 tc.tile_pool(name="t2", bufs=3) as t2p, \
                 tc.tile_pool(name="tcl", bufs=3) as tclp:

                # layer-0 input+state, fp8, interleaved per t-block as two
                # 512-col halves [x(256)|h(256)] so each half is directly a
                # DoubleRow moving operand; h(t) is written into block t+1.
                x0res = [x0resp.tile([P, T * 1024], fp8, tag="x0r", name=f"x0res_{j}")
                         for j in range(NJ)]

                def conv_gather_run(d, tab, col0, nchunk, agg, first):
                    # Gather chunks in groups of 8; m-outer matmuls keep each
                    # PSUM bank's accumulation group sequential (interleaved
                    # groups wedge the exec unit).
                    for g0 in range(0, nchunk, 8):
                        gn = min(8, nchunk - g0)
                        gs, ss = [], []
                        for ki in range(gn):
                            col = col0 + g0 + ki
                            g = gatgp.tile([P, T * F], bf16, tag="gg", name=f"g_{d}_{col}")
                            nc.gpsimd.indirect_dma_start(
                                out=g[:], out_offset=None, in_=tab.ap(),
                                in_offset=bass.IndirectOffsetOnAxis(
                                    ap=esrc_sb[:, col:col + 1], axis=0))
                            s = ssbp.tile([P, P], bf16, tag="s", name=f"s_{d}_{col}")
                            nc.vector.tensor_scalar(
                                out=s[:], in0=iota_sb[:],
                                scalar1=edst_sb[:, col:col + 1],
                                scalar2=ew_sb[:, col:col + 1],
                                op0=ALU.is_equal, op1=ALU.mult)
                            gs.append(g)
                            ss.append(s)
                        pp = psump.tile([P, T * H], f32, tag="ps",
                                        name=f"pp_{d}_{col0 + g0}")
                        for m in range(8):
                            for kk in range(gn):
                                nc.tensor.matmul(
                                    out=pp[:, m * P:(m + 1) * P],
                                    lhsT=gs[kk][:, m * P:(m + 1) * P],
                                    rhs=ss[kk][:], start=(kk == 0), stop=(kk == gn - 1))
                        if first:
                            nc.vector.tensor_copy(agg[:], pp[:, 0:T * F])
                            first = False
                        else:
                            nc.vector.tensor_tensor(out=agg[:], in0=agg[:],
                                                    in1=pp[:, 0:T * F], op=ALU.add)

                def conv_pod_chunks(d, col0, nchunk, agg, first):
                    # Pod conv via one-hot matmuls against the SBUF-resident
                    # node table: C[s,r] = sum_e w_e [src=s][dst=r], then
                    # agg = ntab.T @ C. No DRAM gather at all.
                    ohs, s0s = [], []
                    for ki in range(nchunk):
                        col = col0 + ki
                        oh = gatp.tile([P, 512], bf16, tag="g", name=f"oh_{d}_{ki}")
                        nc.vector.tensor_scalar(
                            out=oh[:], in0=iota512_sb[:],
                            scalar1=esrcf_sb[:, col:col + 1], scalar2=ew_sb[:, col:col + 1],
                            op0=ALU.is_equal, op1=ALU.mult)
                        s0 = ssbp.tile([P, P], bf16, tag="s", name=f"s0_{d}_{ki}")
                        nc.vector.tensor_scalar(
                            out=s0[:], in0=iota_sb[:],
                            scalar1=edst_sb[:, col:col + 1], scalar2=None,
                            op0=ALU.is_equal)
                        ohs.append(oh)
                        s0s.append(s0)
                    cps = psump.tile([P, T * H], f32, tag="ps", name=f"cps_{d}")
                    for sc in range(4):
                        for ki in range(nchunk):
                            nc.tensor.matmul(
                                out=cps[:, sc * P:(sc + 1) * P],
                                lhsT=ohs[ki][:, sc * P:(sc + 1) * P], rhs=s0s[ki][:],
                                start=(ki == 0), stop=(ki == nchunk - 1))
                    csb = ssbp.tile([P, 512], bf16, tag="csb", name=f"csb_{d}")
                    nc.vector.tensor_copy(csb[:], cps[:, 0:512])
                    # reuse the same PSUM tile for the projection (start=True)
                    for m in range(8):
                        for sc in range(4):
                            nc.tensor.matmul(
                                out=cps[:, m * P:(m + 1) * P],
                                lhsT=ntab_sb[sc][:, m * P:(m + 1) * P],
                                rhs=csb[:, sc * P:(sc + 1) * P],
                                start=(sc == 0), stop=(sc == 3))
                    if first:
                        nc.vector.tensor_copy(agg[:], cps[:, 0:T * F])
                    else:
                        nc.vector.tensor_tensor(out=agg[:], in0=agg[:],
                                                in1=cps[:, 0:T * F], op=ALU.add)

                def conv_tile(d):
                    agg = aggsbp.tile([P, T * F], bf16, tag="agg", name=f"agg_{d}")
                    first = True
                    col = base[d]
                    kpod, ksvc, knode = K[d]
                    if kpod:
                        conv_pod_chunks(d, col, kpod, agg, first)
                        first = False
                        col += kpod
                    for kind, nchunk in ((KIND_SVC, ksvc), (KIND_NODE, knode)):
                        if not nchunk:
                            continue
                        conv_gather_run(d, srctabs[kind], col, nchunk, agg, first)
                        first = False
                        col += nchunk

                    # tail: per-segment linear + bias -> PSUM, then LeakyReLU
                    hx = psump.tile([P, T * H], f32, tag="ps", name=f"hx_{d}")
                    for t in range(T):
                        pb = 64 * (t % 2)
                        for kind, r0, r1 in segments[d]:
                            knm = KINDNM[kind]
                            nc.tensor.matmul(
                                out=hx[:, t * H + r0:t * H + r1],
                                lhsT=bt_sb[knm][0:1, t * H:(t + 1) * H],
                                rhs=ones_sb[0:1, 0:r1 - r0], start=True, stop=False)
                            nc.tensor.matmul(
                                out=hx[:, t * H + r0:t * H + r1],
                                lhsT=wt_sb[knm][pb:pb + F, t * H:(t + 1) * H],
                                rhs=agg[pb:pb + F, (t // 2) * P + r0:(t // 2) * P + r1],
                                start=False, stop=True)
                    r = d * P
                    j, rl = r // 512, r % 512
                    # t-block layout: [x (512) | h (512)]
                    dst = x0res[j][:].rearrange("h (t s) -> h t s", t=T)[:, :, rl:rl + P]
                    nc.scalar.activation(
                        dst, hx[:].rearrange("h (t r) -> h t r", t=T), AF.Prelu, alpha=0.01)

                # ---------------- LSTM phase ----------------
                # Three independent 512-row recurrence chains; h double-buffered
                # by timestep parity so the output DMA needs no copy.
                B = 512
                h = [[[hcp.tile([P, B], bf16, tag="hc", name=f"h_{l}_{j}_{p}")
                       for p in range(2)] for j in range(NJ)] for l in range(2)]
                c = [[hcp.tile([P, B], f32, tag="hc", name=f"c_{l}_{j}")
                      for j in range(NJ)] for l in range(2)]

                def cell_mm_sig(l, t, j):
                    gates = psump.tile([P, 4 * B], f32, tag="ps",
                                       name=f"gates_{l}_{j}_{t}")
                    if l == 0:
                        # fp8 DoubleRow path: stride-512 [2,256] moving APs
                        # pair x with h, fusing the K=256 [x;h] contraction
                        # at 0.5 cycles/row
                        xq = x0res[j][:, t * 1024:(t + 1) * 1024].rearrange(
                            "h (xh q) -> h xh q", xh=2)
                        wv = wdr0_sb[:].rearrange("h (two q) -> h two q", two=2)
                        for i in range(4):
                            nc.tensor.matmul(
                                out=gates[:, i * B:(i + 1) * B],
                                lhsT=bias_sb[0][0:1, i * H:(i + 1) * H],
                                rhs=ones_sb[0:1, 0:B], start=True, stop=False,
                                skip_group_check=True)
                            if t == 0:
                                nc.tensor.matmul(
                                    out=gates[:, i * B:(i + 1) * B],
                                    lhsT=wv[:, 0:1, i * H:(i + 1) * H],
                                    rhs=x0res[j][:, 0:512], start=False, stop=True,
                                    skip_group_check=True)
                            else:
                                for hf in range(2):
                                    nc.tensor.matmul(
                                        out=gates[:, i * B + hf * 256:i * B + (hf + 1) * 256],
                                        lhsT=wv[:, :, i * H:(i + 1) * H],
                                        rhs=xq[:, :, hf * 256:(hf + 1) * 256],
                                        start=False, stop=True,
                                        perf_mode=mybir.MatmulPerfMode.DoubleRow,
                                        skip_group_check=True)
                    else:
                        xsrc = h[0][j][t % 2][:]
                        for i in range(4):
                            nc.tensor.matmul(
                                out=gates[:, i * B:(i + 1) * B],
                                lhsT=bias_sb[1][0:1, i * H:(i + 1) * H],
                                rhs=ones_sb[0:1, 0:B], start=True, stop=False)
                            nc.tensor.matmul(
                                out=gates[:, i * B:(i + 1) * B],
                                lhsT=wih1_sb[:, i * H:(i + 1) * H],
                                rhs=xsrc, start=False, stop=(t == 0))
                            if t > 0:
                                nc.tensor.matmul(
                                    out=gates[:, i * B:(i + 1) * B],
                                    lhsT=whh1_sb[:, i * H:(i + 1) * H],
                                    rhs=h[1][j][(t - 1) % 2][:], start=False, stop=True)
                    sg = sgp.tile([P, 4 * B], bf16, tag="sg", name=f"sg_{l}_{j}_{t}")
                    nc.scalar.activation(sg[:], gates[:], AF.Sigmoid)
                    return sg

                def cell_cupd(l, t, j, sg):
                    # gates [i,f,o,2g]; tanh(g) = 2*sigmoid(2g)-1
                    gt = gtp.tile([P, B], bf16, tag="gt", name=f"gt_{l}_{j}_{t}")
                    nc.vector.tensor_scalar(
                        out=gt[:], in0=sg[:, 3 * B:4 * B], scalar1=2.0, scalar2=-1.0,
                        op0=ALU.mult, op1=ALU.add)
                    if t == 0:
                        nc.vector.tensor_mul(c[l][j][:], sg[:, 0:B], gt[:])
                    else:
                        t1 = t1p.tile([P, B], f32, tag="t1", name=f"t1_{l}_{j}_{t}")
                        nc.vector.tensor_mul(t1[:], sg[:, B:2 * B], c[l][j][:])
                        t2 = t2p.tile([P, B], bf16, tag="t2", name=f"t2_{l}_{j}_{t}")
                        nc.vector.tensor_mul(t2[:], sg[:, 0:B], gt[:])
                        nc.gpsimd.tensor_tensor(c[l][j][:], t1[:], t2[:], op=ALU.add)

                def cell_tail(l, t, j, sg):
                    tcl = tclp.tile([P, B], bf16, tag="tc", name=f"tc_{l}_{j}_{t}")
                    nc.scalar.activation(tcl[:], c[l][j][:], AF.Tanh)
                    nc.vector.tensor_mul(h[l][j][t % 2][:], sg[:, 2 * B:3 * B], tcl[:])
                    if l == 0 and t + 1 < T:
                        # fp8 copy of h into block t+1's h-half for DoubleRow
                        hv = x0res[j][:].rearrange("h (t s) -> h t s", t=T)
                        nc.vector.tensor_mul(hv[:, t + 1:t + 2, 512:1024],
                                             sg[:, 2 * B:3 * B], tcl[:])
                    if l == 1:
                        nc.sync.dma_start(
                            out_d.ap()[:, t * R_CORE + j * B:t * R_CORE + (j + 1) * B],
                            h[1][j][t % 2][:])

                # Ragged software pipeline: LSTM tile j starts its recurrence
                # O[j] steps in, as soon as its 4 conv tiles are done; the
                # remaining conv tiles interleave with the early LSTM steps so
                # conv PE/DMA hides under LSTM ScalarE/DVE work.
                O = [3 * j for j in range(NJ)]
                conv_at = {}
                for j in range(1, NJ):
                    tiles = list(range(4 * j, 4 * j + 4))
                    steps = [O[j] - 3, O[j] - 3, O[j] - 2, O[j] - 1]
                    for d, s in zip(tiles, steps):
                        conv_at.setdefault(s, []).append(d)
                for d in range(min(4, N_TILES)):
                    conv_tile(d)
                for s in range(T + (O[-1] if NJ else 0)):
                    for d in conv_at.get(s, []):
                        conv_tile(d)
                    for l in range(2):
                        js = [j for j in range(NJ) if 0 <= s - O[j] < T]
                        sgs = {j: cell_mm_sig(l, s - O[j], j) for j in js}
                        for j in js:
                            cell_cupd(l, s - O[j], j, sgs[j])
                        for j in js:
                            cell_tail(l, s - O[j], j, sgs[j])

    nc.compile()
    return nc


# ----------------------------------------------------------------------------
# Entry points
# ----------------------------------------------------------------------------

def _assemble(results, meta, maps):
    (C_total, K, PN, PS, POD_PC, SVC_PC) = meta
    pod_map, svc_map, n_pod_u, n_svc_u = maps
    ROWS = POD_PC + SVC_PC + NODE_PC
    R_CORE = 512 * ((ROWS + 511) // 512)
    pod_u = np.empty((NCORES * POD_PC, T, H), dtype=np.float32)
    svc_u = np.empty((NCORES * SVC_PC, T, H), dtype=np.float32)
    node_u = np.empty((NCORES * NODE_PC, T, H), dtype=np.float32)
    s0, n0 = POD_PC, POD_PC + SVC_PC
    for cidx, res in enumerate(results):
        o = res["out"].astype(np.float32).reshape(H, T, R_CORE).transpose(2, 1, 0)
        pod_u[cidx * POD_PC:(cidx + 1) * POD_PC] = o[0:POD_PC]
        svc_u[cidx * SVC_PC:(cidx + 1) * SVC_PC] = o[s0:s0 + SVC_PC]
        node_u[cidx * NODE_PC:(cidx + 1) * NODE_PC] = o[n0:n0 + NODE_PC]
    full = np.empty((N_NODE + N_POD + N_SVC, T, H), dtype=np.float32)
    full[0:N_NODE] = node_u[0:N_NODE]
    full[N_NODE:N_NODE + N_POD] = pod_u[pod_map]
    full[N_NODE + N_POD:] = svc_u[svc_map]
    return full


def run(inputs, trace=False):
    from concourse.bass_utils import run_bass_kernel_spmd
    meta, maps, in_maps = _prep(inputs)
    if meta not in _COMPILED:
        _COMPILED[meta] = _build(meta)
    nc = _COMPILED[meta]
    try:
        res = run_bass_kernel_spmd(nc, in_maps, core_ids=list(range(NCORES)), trace=trace)
    except Exception:
        # transient device errors recover on re-execution; retry once
        res = run_bass_kernel_spmd(nc, in_maps, core_ids=list(range(NCORES)), trace=trace)
    return _assemble(res.results, meta, maps), res


def kernel(**inputs):
    out, _ = run(inputs, trace=False)
    return out
